# revision 19
# baseline (speedup 1.0000x reference)
"""TRN2 Bass kernel for nn_CombCrossAttention (GQA cross-attention block).

Computation (T=2048, K=2048, E=4096, H=32 q-heads, KVH=8 kv-heads, D=128):
    q  = hidden @ Wq.T;  per-head RMSNorm(q) * q_norm_w
    kn = RMSNorm(k) * k_norm_w  (GQA: each kv head serves 4 q heads)
    attn = softmax(qn @ kn.T / sqrt(D)) @ v
    out  = attn @ Wo.T

Sharding: tensor-parallel over heads on 8 NeuronCores. Core c owns q-heads
4c..4c+3 (Wq rows 512c..512c+512) and kv-head c, plus Wo columns
512c..512c+512; each core emits a [T, E] partial of the o-projection (bf16)
and the host sums the 8 partials (the "all-reduce").

Everything is computed transposed ([feature, t]); all matmul inputs bf16.
The kernel is a 6-iteration software pipeline over the four 512-column
t-blocks (tcn): iteration i runs P1(i) q-proj+RMSNorm, P2(i-1) attention,
P3(i-2) o-proj, with instructions woven at ~2.5us step granularity so the
PE instruction queue (in-order) never stalls behind the scalar-engine exp
chain. PSUM budget (8 banks): pq 2 + ps 1 + pscr 2 + po 2 + pout 1.
P2 processes heads in 2-head halves (g) so po fits in 2 banks. The
softmax denominator accumulates exp tiles elementwise on DVE (g=0) and
Pool (g=1), finished by a ones-matmul partition sum; 1/denom uses the
fast approximate reciprocal. exp() needs no max-subtraction: post-RMSNorm
scores are O(1)-bounded. hid streams per-tcn (double buffered) over
parallel DMA queues; PSUM drains alternate DVE/Pool.
"""
import sys

sys.path.insert(0, "/opt/trn_rl_repo")

import numpy as np
import ml_dtypes

import jax
try:
    jax.config.update("jax_compilation_cache_dir", "/tmp/jax_neff_cache")
    jax.config.update("jax_persistent_cache_min_compile_time_secs", 1.0)
except Exception:
    pass

import concourse.bass as bass  # noqa: F401
import concourse.hw_specs as hw_specs
import concourse.mybir as mybir
import concourse.tile as tile
from concourse import bacc, bass_utils


def _force_combined_act_table(arch):
    """Make Ln/Exp resolvable only via the table that contains BOTH, so the
    act-table-load pass never thrashes between natural_log and exp_and_others
    (each reload is 1283ns on the scalar engine, in the softmax critical
    chain). Mutates the cached table sets; indices into act_info.json are
    unchanged, so the emitted act_func_set_id stays valid for walrus."""
    try:
        tabs = hw_specs.get_activation_tables(arch)
    except Exception:
        return
    ln_t, exp_t = (mybir.ActivationFunctionType.Ln,
                   mybir.ActivationFunctionType.Exp)
    combined = [n for n, s in tabs.items() if ln_t in s and exp_t in s]
    if not combined:
        return
    keep = combined[0]
    for n, s in tabs.items():
        if n != keep:
            s.discard(ln_t)
            s.discard(exp_t)

EPS = 1e-5
T, K, E, H, KVH, D = 2048, 2048, 4096, 32, 8, 128
N_CORES = 8
HL = H // N_CORES      # 4 q-heads per core
EL = HL * D            # 512 local embed rows/cols
f32 = mybir.dt.float32
f32r = mybir.dt.float32r
bf16 = mybir.dt.bfloat16

Ln = mybir.ActivationFunctionType.Ln
Exp = mybir.ActivationFunctionType.Exp

NT = 4   # t-blocks of 512
NKK = 16  # key tiles of 128
NKE = 32  # embed contraction tiles of 128


def _weave(*streams):
    """Round-robin the step lists; each step is a list of thunks."""
    n = max(len(s) for s in streams)
    for slot in range(n):
        for s in streams:
            if slot < len(s):
                for thunk in s[slot]:
                    thunk()


def _kernel_body(tc):
    nc = tc.nc
    hid = nc.dram_tensor("hid", [NT, 8, 128, 4, 512], bf16,
                         kind="ExternalInput").ap()
    wq = nc.dram_tensor("wq", [HL, 128, NKE, 128], bf16,
                        kind="ExternalInput").ap()
    kpp = nc.dram_tensor("kpp", [128, NKK, 128], bf16,
                         kind="ExternalInput").ap()
    vt = nc.dram_tensor("vt", [128, NKK, 128], bf16,
                        kind="ExternalInput").ap()
    wo = nc.dram_tensor("wo", [128, 4, 32, 128], bf16,
                        kind="ExternalInput").ap()
    onesf = nc.dram_tensor("onesf", [128, 128], f32r,
                           kind="ExternalInput").ap()
    onesb = nc.dram_tensor("onesb", [128, 128], bf16,
                           kind="ExternalInput").ap()
    outp = nc.dram_tensor("outp", [NT, 16, 128, 2, 512], bf16,
                          kind="ExternalOutput").ap()

    with tc.tile_pool(name="persist", bufs=1) as persist, \
         tc.tile_pool(name="hidp", bufs=2) as hidp, \
         tc.tile_pool(name="wqp", bufs=2) as wqp, \
         tc.tile_pool(name="qtp", bufs=2) as qtp, \
         tc.tile_pool(name="aop", bufs=2) as aop, \
         tc.tile_pool(name="sqp", bufs=2) as sqp, \
         tc.tile_pool(name="srp", bufs=2) as srp, \
         tc.tile_pool(name="exp_", bufs=4) as exp_, \
         tc.tile_pool(name="eap", bufs=2) as eap, \
         tc.tile_pool(name="rdp", bufs=2) as rdp, \
         tc.tile_pool(name="obp", bufs=3) as obp, \
         tc.tile_pool(name="pqs", bufs=1, space="PSUM") as pqs, \
         tc.tile_pool(name="pss", bufs=1, space="PSUM") as pss, \
         tc.tile_pool(name="scs", bufs=1, space="PSUM") as scs, \
         tc.tile_pool(name="pos", bufs=1, space="PSUM") as pos, \
         tc.tile_pool(name="pts", bufs=2, space="PSUM") as pts:

        # Startup-critical DMAs first (wq0 + hid tcn0 gate the first matmul);
        # everything else behind them or woven into iteration 0.
        eps_col = persist.tile([128, 1], f32)
        nc.vector.memset(eps_col, EPS)
        ones = persist.tile([128, 128], f32r)
        ones_b = persist.tile([128, 128], bf16)
        k_sb = persist.tile([128, NKK, 128], bf16)
        v_sb = persist.tile([128, NKK, 128], bf16)
        wo_sb = persist.tile([128, 4, 32, 128], bf16)

        def preload_rest():
            nc.gpsimd.dma_start(out=k_sb, in_=kpp)
            nc.gpsimd.dma_start(out=ones, in_=onesf)
            nc.gpsimd.dma_start(out=ones_b, in_=onesb)
            nc.sync.dma_start(out=v_sb, in_=vt)

        def preload_wo():
            nc.sync.dma_start(out=wo_sb, in_=wo)

        hid_t = {}   # tcn -> SBUF tile [128, 32, 512]
        wq_m = {}    # (tcn, m) -> SBUF tile [128, 32, 128]
        qT_t = {}    # tcn -> [128, HL, 512]
        aoT_t = {}   # tcn -> [128, HL, 512]

        def dma_hid(t):
            """8 DMAs of 4 k-chunks each, alternating sync/gpsimd."""
            hid_t[t] = hidp.tile([128, NKE, 512], bf16, tag="hid",
                                 name=f"hid{t}")
            steps = []
            for g in range(8):
                eng = [nc.sync, nc.gpsimd][g % 2]
                steps.append(lambda t=t, g=g, eng=eng: eng.dma_start(
                    out=hid_t[t][:, 4 * g:4 * g + 4, :], in_=hid[t, g]))
            return steps

        def dma_wq(t, m):
            wq_m[(t, m)] = wqp.tile([128, NKE, 128], bf16, tag="wqm",
                                    name=f"wq{t}_{m}")
            return lambda t=t, m=m: nc.sync.dma_start(
                out=wq_m[(t, m)], in_=wq[m])

        # ---------------- P1(t): q-proj + RMSNorm -> qT_t[t] --------------
        def p1_steps(t):
            qT_t[t] = qtp.tile([128, HL, 512], bf16, tag="qt", name=f"qt{t}")
            steps = []
            for m in range(HL):
                pq = pqs.tile([128, 512], f32, tag="pq", name=f"pq{t}_{m}")
                for grp in range(8):
                    th = []
                    if grp == 2 and m < HL - 1:
                        th.append(dma_wq(t, m + 1))
                    if grp == 5 and m == HL - 1 and t < NT - 1:
                        th.append(dma_wq(t + 1, 0))

                    def mm4(t=t, m=m, grp=grp, pq=pq):
                        for k in range(4 * grp, 4 * grp + 4):
                            nc.tensor.matmul(pq, wq_m[(t, m)][:, k, :],
                                             hid_t[t][:, k, :],
                                             start=(k == 0), stop=(k == 31))
                    th.append(mm4)
                    steps.append(th)
                sq = sqp.tile([128, 512], f32r, tag="sq")
                ps = pss.tile([128, 512], f32, tag="ps")
                lns = srp.tile([128, 512], f32, tag="ln")
                ri = srp.tile([128, 512], f32, tag="ri")

                def s_sq(sq=sq, pq=pq):
                    nc.scalar.square(sq, pq)

                def s_ones(ps=ps, sq=sq, lns=lns):
                    nc.tensor.matmul(ps, ones, sq, start=True, stop=True)
                    nc.scalar.activation(lns, ps, Ln, scale=1.0 / D,
                                         bias=eps_col[:])

                def s_ri(ri=ri, lns=lns, t=t, m=m, pq=pq):
                    nc.scalar.activation(ri, lns, Exp, scale=-0.5)
                    nc.vector.tensor_mul(qT_t[t][:, m, :], pq, ri)
                steps.append([s_sq])
                steps.append([s_ones])
                steps.append([s_ri])
            return steps

    # ---------------- P2(t, g): one 2-head half of attention -------------
        def p2_half(t, g):
            if g == 0:
                aoT_t[t] = aop.tile([128, HL, 512], bf16, tag="ao",
                                    name=f"ao{t}")
            steps = []
            po = pos.tile([128, 2, 512], f32, tag="po", name=f"po{t}_{g}")
            exacc = eap.tile([128, 2, 512], bf16, tag="ea", name=f"ea{t}_{g}")
            acc_eng = nc.vector  # Pool is ~3x slower on these; keep DVE
            prev = [None]
            for kk in range(NKK):
                th = []
                if kk > 0:
                    def attnv(t=t, g=g, kk=kk - 1, po=po, ex=prev[0],
                              exacc=exacc, acc_eng=acc_eng):
                        for j in range(2):
                            nc.tensor.matmul(po[:, j, :], v_sb[:, kk, :],
                                             ex[:, j, :],
                                             start=(kk == 0),
                                             stop=(kk == NKK - 1))
                        if kk == 0:
                            acc_eng.tensor_copy(exacc, ex)
                        else:
                            acc_eng.tensor_add(exacc, exacc, ex)
                    th.append(attnv)
                pscr = scs.tile([128, 2, 512], f32, tag="sc",
                                name=f"sc{t}_{g}_{kk}")
                ex = exp_.tile([128, 2, 512], bf16, tag="ex",
                               name=f"ex{t}_{g}_{kk}")
                prev[0] = ex

                def scores(t=t, g=g, kk=kk, pscr=pscr, ex=ex):
                    for j in range(2):
                        nc.tensor.matmul(pscr[:, j, :], k_sb[:, kk, :],
                                         qT_t[t][:, 2 * g + j, :],
                                         start=True, stop=True)
                    nc.scalar.activation(ex, pscr, Exp)
                th.append(scores)
                steps.append(th)

            def attnv_last(t=t, g=g, po=po, ex=prev[0], exacc=exacc,
                           acc_eng=acc_eng):
                kk = NKK - 1
                for j in range(2):
                    nc.tensor.matmul(po[:, j, :], v_sb[:, kk, :],
                                     ex[:, j, :], start=False, stop=True)
                acc_eng.tensor_add(exacc, exacc, ex)
            steps.append([attnv_last])
            pd = scs.tile([128, 2, 512], f32, tag="sc", name=f"pd{t}_{g}")
            rd = rdp.tile([128, 2, 512], f32, tag="rd")

            def denom(pd=pd, exacc=exacc, rd=rd):
                for j in range(2):
                    nc.tensor.matmul(pd[:, j, :], ones_b, exacc[:, j, :],
                                     start=True, stop=True)
                nc.vector.reciprocal_approx_fast(out=rd, in_=pd)

            def aomul(t=t, g=g, po=po, rd=rd):
                nc.vector.tensor_mul(aoT_t[t][:, 2 * g:2 * g + 2, :],
                                     po, rd)
            steps.append([denom])
            steps.append([aomul])
            return steps

        # ---------------- P3(t): o-proj partial -> outp[t] ----------------
        def p3_steps(t):
            steps = []
            ob = [None]
            for m in range(32):
                th = []
                pout = pts.tile([128, 512], f32, tag="pp", name=f"pp{t}_{m}")
                if m % 2 == 0:
                    def alloc_ob(m=m):
                        ob[0] = obp.tile([128, 2, 512], bf16, tag="ob",
                                         name=f"ob{t}_{m}")
                    th.append(alloc_ob)

                def mm(t=t, m=m, pout=pout):
                    for k in range(4):
                        nc.tensor.matmul(pout, wo_sb[:, k, m, :],
                                         aoT_t[t][:, k, :],
                                         start=(k == 0), stop=(k == 3))
                    # GPSIMD cannot read PSUM; scalar paces exp: drain on DVE.
                    nc.vector.tensor_copy(ob[0][:, m % 2, :], pout)
                th.append(mm)
                if m % 2 == 1:
                    def dma_out(t=t, m=m):
                        eng = [nc.sync, nc.gpsimd][(m // 2) % 2]
                        eng.dma_start(out=outp[t, m // 2], in_=ob[0])
                    th.append(dma_out)
                steps.append(th)
            return steps

        # ---------------- pipeline ----------------------------------------
        # Startup: first wq tile in 4 chunks and hid(0) woven across the two
        # DMA queues so the first matmul can launch after ~2 small transfers.
        wq_m[(0, 0)] = wqp.tile([128, NKE, 128], bf16, tag="wqm", name="wq0_0")
        hid_t[0] = hidp.tile([128, NKE, 512], bf16, tag="hid", name="hid0")
        engs = [nc.sync, nc.gpsimd]
        for c in range(4):
            engs[c % 2].dma_start(out=wq_m[(0, 0)][:, 8 * c:8 * c + 8, :],
                                  in_=wq[0][:, 8 * c:8 * c + 8, :])
            engs[c % 2].dma_start(out=hid_t[0][:, 8 * c:8 * c + 4, :],
                                  in_=hid[0, 2 * c])
            engs[(c + 1) % 2].dma_start(
                out=hid_t[0][:, 8 * c + 4:8 * c + 8, :], in_=hid[0, 2 * c + 1])
        preload_rest()
        for i in range(NT + 2):
            streams = []
            if i < NT:
                s1 = p1_steps(i)
                if i == 0:
                    s1[12].append(preload_wo)
                if i + 1 < NT:
                    # prefetch next tcn's hid, spread through the iteration
                    for si, th in enumerate(dma_hid(i + 1)):
                        s1[min(3 + 4 * si, len(s1) - 1)].append(th)
                streams.append(s1)
            if i >= 2:
                streams.append(p3_steps(i - 2))
            # P2 phased by half: g1 of the previous tcn runs in the front of
            # this iteration; g0 of the current tcn in the back half (its qT
            # heads 0-1 are ready mid-iteration via subtile deps). P2 last
            # in each slot so its scores' PSUM-WAR wait has matmuls ahead.
            if 1 <= i <= NT:
                streams.append(p2_half(i - 1, 1))
            if i < NT:
                streams.append([[]] * 24 + p2_half(i, 0))
            _weave(*streams)


_NC_CACHE = None


def _build():
    global _NC_CACHE
    if _NC_CACHE is None:
        nc = bacc.Bacc("TRN2", target_bir_lowering=False, debug=False,
                       num_devices=N_CORES)
        _force_combined_act_table(nc.m.arch)
        with tile.TileContext(nc) as tc:
            _kernel_body(tc)
        nc.compile()
        _NC_CACHE = nc
    return _NC_CACHE


def _prepare_in_maps(hidden_states, k, v, Wq, Wo, q_norm_w, k_norm_w):
    bf = ml_dtypes.bfloat16
    hs = np.asarray(hidden_states, np.float32)
    k_ = np.asarray(k, np.float32)[0]      # [K, KVH, D]
    v_ = np.asarray(v, np.float32)[0]
    Wq_ = np.asarray(Wq, np.float32)
    Wo_ = np.asarray(Wo, np.float32)
    wqn = np.asarray(q_norm_w, np.float64)
    wkn = np.asarray(k_norm_w, np.float64)

    # Fold k-RMSNorm, both norm weights, and the attention scale into k''.
    kd = k_.astype(np.float64)
    rk = 1.0 / np.sqrt((kd ** 2).mean(-1, keepdims=True) + EPS)
    kpp_full = (kd * rk * (wqn * wkn) * (D ** -0.5)).astype(np.float32)

    hidT = np.ascontiguousarray(hs.T)                                  # [E, T]
    # [t, g, p, kg, c]: hid chunk layout, e = 128*(4g+kg)+p, t' = 512t+c
    hid_tiles = np.ascontiguousarray(
        hidT.reshape(8, 4, 128, 4, 512).transpose(3, 0, 2, 1, 4).astype(bf))
    onesf_arr = np.ones((128, 128), np.float32)
    onesb_arr = np.ones((128, 128), bf)

    in_maps = []
    for c in range(N_CORES):
        wqT = np.ascontiguousarray(Wq_[c * EL:(c + 1) * EL, :].T)      # [E, EL]
        # [m, p, k, col]: lhsT tiles for head m, partition-major like SBUF
        wq_tiles = np.ascontiguousarray(
            wqT.reshape(32, 128, 4, 128).transpose(2, 1, 0, 3).astype(bf))
        woT = np.ascontiguousarray(Wo_[:, c * EL:(c + 1) * EL].T)      # [EL, E]
        wo_tiles = np.ascontiguousarray(
            woT.reshape(4, 128, 32, 128).transpose(1, 0, 2, 3).astype(bf))
        kppT = np.ascontiguousarray(kpp_full[:, c, :].T)               # [D, K]
        kpp_tiles = np.ascontiguousarray(
            kppT.reshape(128, 16, 128).astype(bf))
        v_tiles = np.ascontiguousarray(
            v_[:, c, :].reshape(16, 128, 128).transpose(1, 0, 2).astype(bf))
        in_maps.append({
            "hid": hid_tiles, "wq": wq_tiles, "kpp": kpp_tiles,
            "vt": v_tiles, "wo": wo_tiles,
            "onesf": onesf_arr, "onesb": onesb_arr,
        })
    return in_maps


def _gather(results):
    total = np.zeros((E, T), np.float64)
    for r in results:
        # outp [t, mg, p, j, c]: e = 128*(2mg+j)+p, t' = 512t+c
        part = r["outp"].astype(np.float32).transpose(1, 3, 2, 0, 4)
        total += part.reshape(E, T)
    return np.ascontiguousarray(total.T.astype(np.float32))


def kernel(hidden_states, k, v, Wq, Wo, q_norm_w, k_norm_w):
    nc = _build()
    in_maps = _prepare_in_maps(hidden_states, k, v, Wq, Wo, q_norm_w, k_norm_w)
    res = bass_utils.run_bass_kernel_spmd(nc, in_maps,
                                          core_ids=list(range(N_CORES)))
    return _gather(res.results)
